# revision 1
# baseline (speedup 1.0000x reference)
"""Distributed AQT int8 fake-quant matmul on 8 Trainium2 NeuronCores.

Computes reference:
    lhs_q = fake_quant_int8(lhs); rhs_q = fake_quant_int8(rhs)
    out = lhs_q @ rhs_q            # [4096, 8192] f32

Sharding: 2x4 core grid. Core (i,j) computes the [2048, 2048] output block
(M-half i, N-quarter j) as a K=2048 matmul.

Per the sharding hint, the per-tensor scale is replicated: the global absmax
scale (2 scalars) is computed on host in f32 (bit-identical to the reference
reduction, which is order-independent) and baked into the program as
immediates; each device quantizes its shard locally.

Quantized values (ints in [-127,127]) are exact in bf16, so the matmul runs
at full bf16 PE rate and the result matches the f32 fake-quant reference to
~1e-6. Rounding uses the magic-constant trick: bf16(f32(x*s + 1.5*2^23) - C)
== round-half-even(x*s), bit-identical to jnp.round.

Pipeline (per core): stream f32 shards once; quantize on ACT (x*s+C) + DVE
(-C, cast bf16) into persistent SBUF caches; 1024 bf16 matmuls in 16 waves
(one 128-row m-tile x full N=2048 per wave, 4 PSUM banks, one weight load
per 4 matmuls); dequantized PSUM evacuation on ACT; outputs DMAed via
gpsimd so input DMAs (sync) are never queued behind them (the final waves
use the by-then-idle sync queue to shorten the tail). The first two rhs
k-rows are quantized in 512-col chunks so the first matmul starts as soon
as ~256KB have streamed in.
"""

import numpy as np

import concourse.bass as bass
import concourse.mybir as mybir
import concourse.tile as tile
from concourse import bacc
from concourse.bass_utils import run_bass_kernel_spmd

# Problem shape (hardcoded per contract)
M_FULL, K, N_FULL = 4096, 2048, 8192
RI, CJ = 2, 4                      # core grid: M shards x N shards
M, N = M_FULL // RI, N_FULL // CJ  # 2048 x 2048 per-core output block
P = 128
KT = K // P                        # 16 k-tiles
MT = M // P                        # 16 m-tiles (one wave each)
NB = N // 512                      # 4 n-blocks of 512
C_MAGIC = 12582912.0               # 1.5 * 2^23
CLIP = 127.0
NCORES = RI * CJ

F32 = mybir.dt.float32
BF16 = mybir.dt.bfloat16
AF = mybir.ActivationFunctionType

# tuning knobs
STN_BUFS = 6   # [P,2048] f32 input staging (rhs k-rows)
STM_BUFS = 10  # [P,512] f32 input staging (lhsT chunks)
OST_BUFS = 4   # [P,512] f32 output staging
SYNC_OUT_WAVES = 3  # trailing waves whose outputs use the idle sync queue


def _build_nc(s_l, s_r, d_q):
    nc = bacc.Bacc("TRN2", target_bir_lowering=False, debug=False,
                   num_devices=NCORES)
    lhsT = nc.dram_tensor("lhsT", [K, M], F32, kind="ExternalInput")
    rhs = nc.dram_tensor("rhs", [K, N], F32, kind="ExternalInput")
    out = nc.dram_tensor("out", [M, N], F32, kind="ExternalOutput")

    with tile.TileContext(nc) as tc:
        _emit(nc, tc, lhsT, rhs, out, s_l, s_r, d_q)
    nc.compile()
    return nc


def _emit(nc, tc, lhsT, rhs, out, s_l, s_r, d_q):
    from contextlib import ExitStack
    ctx = ExitStack()
    with ctx:
        pstn = ctx.enter_context(tc.tile_pool(name="stn", bufs=STN_BUFS))
        pstm = ctx.enter_context(tc.tile_pool(name="stm", bufs=STM_BUFS))
        pcache = ctx.enter_context(tc.tile_pool(name="cache", bufs=1))
        ppsum = ctx.enter_context(tc.tile_pool(name="psum", bufs=8, space="PSUM"))
        post = ctx.enter_context(tc.tile_pool(name="ost", bufs=OST_BUFS))
        pconst = ctx.enter_context(tc.tile_pool(name="const", bufs=1))

        cb = pconst.tile([P, 1], F32, tag="cb")
        nc.vector.memset(cb[:], C_MAGIC)

        # persistent bf16 caches: qn[kt] = full k-row of rhs; qm[kt][c] =
        # 512-col chunk of lhsT (chunk c feeds waves 4c..4c+3)
        qn = [pcache.tile([P, N], BF16, tag=f"qn{kt}", name=f"qn{kt}")
              for kt in range(KT)]
        qm = [[pcache.tile([P, 512], BF16, tag=f"qm{kt}_{c}",
                           name=f"qm{kt}_{c}")
               for c in range(1, 4)] for kt in range(KT)]
        qm0 = [[pcache.tile([P, 256], BF16, tag=f"qm0{kt}_{h}",
                            name=f"qm0{kt}_{h}")
                for h in range(2)] for kt in range(KT)]

        def quant_n(kt, chunks=1):
            st = pstn.tile([P, N], F32, tag="stn")
            w = N // chunks
            for c in range(chunks):
                cs = slice(c * w, (c + 1) * w)
                nc.sync.dma_start(st[:, cs], rhs[kt * P:(kt + 1) * P, cs])
                nc.scalar.activation(st[:, cs], st[:, cs], AF.Identity,
                                     bias=cb[:], scale=float(s_r))
                nc.vector.tensor_scalar_add(qn[kt][:, cs], st[:, cs],
                                            -C_MAGIC)

        def quant_m(kt, c):
            st = pstm.tile([P, 512], F32, tag="stm")
            nc.sync.dma_start(st[:], lhsT[kt * P:(kt + 1) * P,
                                          c * 512:(c + 1) * 512])
            nc.scalar.activation(st[:], st[:], AF.Identity, bias=cb[:],
                                 scale=float(s_l))
            nc.vector.tensor_scalar_add(qm[kt][c - 1][:], st[:], -C_MAGIC)

        def quant_m0(kt, h):
            st = pstm.tile([P, 512], F32, tag="stm")
            s2 = st[:, :256]
            nc.sync.dma_start(s2, lhsT[kt * P:(kt + 1) * P,
                                       h * 256:(h + 1) * 256])
            nc.scalar.activation(s2, s2, AF.Identity, bias=cb[:],
                                 scale=float(s_l))
            nc.vector.tensor_scalar_add(qm0[kt][h][:], s2, -C_MAGIC)

        def wave(mt, sync_out=False):
            psums = [ppsum.tile([P, 512], F32, tag="ps", name=f"ps{mt}_{nb}")
                     for nb in range(NB)]
            for kt in range(KT):
                if mt < 4:
                    w_ap = qm0[kt][mt // 2][:, (mt % 2) * 128:
                                            (mt % 2 + 1) * 128]
                else:
                    w_ap = qm[kt][mt // 4 - 1][:, (mt % 4) * 128:
                                               (mt % 4 + 1) * 128]
                for nb in range(NB):
                    nc.tensor.matmul(psums[nb][:], w_ap,
                                     qn[kt][:, nb * 512:(nb + 1) * 512],
                                     start=(kt == 0), stop=(kt == KT - 1))
            m0 = mt * P
            for nb in range(NB):
                o = post.tile([P, 512], F32, tag="ost")
                nc.scalar.activation(o[:], psums[nb][:], AF.Copy,
                                     scale=float(d_q))
                eng = nc.sync if sync_out else nc.gpsimd
                eng.dma_start(out[m0:m0 + P, nb * 512:(nb + 1) * 512], o[:])

        # emission: quantize chunks are emitted one wave-group ahead of the
        # waves that consume them (their DMAs queue behind the earlier group
        # and land well before the consuming waves start), keeping per-engine
        # FIFO order pipeline-consistent with no group-boundary stalls.
        for kt in range(KT):
            quant_n(kt, chunks=4 if kt < 2 else 1)
            quant_m0(kt, 0)
        wave(0)
        for kt in range(KT):
            quant_m0(kt, 1)
        wave(1)
        for kt in range(KT):
            quant_m(kt, 1)
        wave(2)
        wave(3)
        for g in range(2, 4):
            for w in range(4):
                wave(4 * (g - 1) + w)
                if w < 2:
                    for kt in range(KT // 2):
                        quant_m(kt + (KT // 2) * w, g)
        for w in range(4):
            wave(12 + w, sync_out=(w >= 4 - SYNC_OUT_WAVES))


_NC_CACHE = {}


def _get_nc(s_l, s_r, d_q):
    key = (float(s_l), float(s_r), float(d_q))
    if key not in _NC_CACHE:
        _NC_CACHE[key] = _build_nc(*key)
    return _NC_CACHE[key]


def _host_scales(lhs, rhs):
    # exact mirror of the reference reduction (order-independent in f32)
    ml = np.maximum(np.abs(lhs).max(), np.float32(1e-6))
    mr = np.maximum(np.abs(rhs).max(), np.float32(1e-6))
    s_l = np.float32(CLIP) / ml
    s_r = np.float32(CLIP) / mr
    d_q = (np.float32(1.0) / s_l) * (np.float32(1.0) / s_r)
    return s_l, s_r, d_q


LAST_RESULT = None  # BassKernelResults of the most recent run (for test.py)


def kernel(lhs, rhs, _trace=False, _trace_cores=None):
    global LAST_RESULT
    lhs = np.ascontiguousarray(np.asarray(lhs, dtype=np.float32))
    rhs = np.ascontiguousarray(np.asarray(rhs, dtype=np.float32))
    assert lhs.shape == (M_FULL, K) and rhs.shape == (K, N_FULL)

    lhsT = np.ascontiguousarray(lhs.T)  # [K, M_FULL]
    s_l, s_r, d_q = _host_scales(lhs, rhs)

    in_maps = []
    for i in range(RI):
        lT = np.ascontiguousarray(lhsT[:, i * M:(i + 1) * M])
        for j in range(CJ):
            r = np.ascontiguousarray(rhs[:, j * N:(j + 1) * N])
            in_maps.append({"lhsT": lT, "rhs": r})

    nc = _get_nc(s_l, s_r, d_q)
    res = run_bass_kernel_spmd(
        nc, in_maps, core_ids=list(range(NCORES)),
        trace=_trace,
        **({"trace_cores": _trace_cores} if _trace_cores else {}))
    LAST_RESULT = res

    full = np.empty((M_FULL, N_FULL), dtype=np.float32)
    for i in range(RI):
        for j in range(CJ):
            full[i * M:(i + 1) * M, j * N:(j + 1) * N] = \
                res.results[i * CJ + j]["out"]
    return full

